# revision 5
# baseline (speedup 1.0000x reference)
"""Dinov3 ViT attention kernel for Trainium2 (8 NeuronCores, data-parallel over batch).

Per core: 2 batch items. hidden_states [2*1029, 1024] in, out [2*1029, 1024] f32.

Host pre-casts hidden_states + weights to bf16 (the kernel computes in bf16
internally anyway, so this only halves DMA traffic).

Per item pipeline (PE-dense, interleaved with ACT-bound attention):
  X-prep (strided DMA to feature-major XT) ->
  V-proj chunk 0 (heads 0..7) ->
  for kq in 0..7:  # head pair (2kq, 2kq+1)
    per head: phase A: S^T per key-tile (K=64 matmul) -> exp on ScalarE
              (scale=1/8, no max: |scores| < ~7) into a 9-slot SBUF ring;
              phase B: AV with es as the STATIONARY operand -> out
              [128 queries, 65] per query tile (col 64 = softmax sum via
              ones-augmented V), accumulated over key tiles in one PSUM bank;
              normalize via per-partition reciprocal + tensor_scalar.
    per pair: PE-transpose normalized [q, d] tiles back to feature-major
              AOT, PSUM->SBUF copy on the (otherwise idle) Pool engine.
    (Q/K proj mo+1, RoPE, V chunk 1, prev-item out-proj pumped into gaps.)
  5-query tail batched into one [128,45] PSUM bank + single exp per bank,
  normalized via DVE reciprocal + gpsimd partition_broadcast.
  out-projection emitted transposed: Y^T = Wo^T-stationary @ AOT, bias via
  per-partition tensor_scalar, DMA'd as [H, TOK] f32 (host transposes back).
"""
import sys
import time

sys.path.insert(0, "/opt/trn_rl_repo")

import ml_dtypes
import numpy as np

import concourse.bacc as bacc
import concourse.mybir as mybir
import concourse.tile as tile

f32 = mybir.dt.float32
bf16 = mybir.dt.bfloat16
FP = mybir.ActivationFunctionType
ADD = mybir.AluOpType.add
MUL = mybir.AluOpType.mult

H = 1024
NH = 16
HD = 64
T = 1029
NPREF = 5
PATCH = 1024
B = 16
NCORES = 8
BPC = B // NCORES          # batch items per core
KO = H // 128              # 8 feature k-tiles
TOK = BPC * T              # tokens per core (2058)
SCALE = 1.0 / float(np.sqrt(HD))

TOK_TILES = [(i * 128, min(128, T - i * 128)) for i in range((T + 127) // 128)]
NJT = len(TOK_TILES)                   # 9 key tiles (8 full + 5)
NQT = 8                                # full 128-query tiles (0..1024)
QCHUNKS = [(0, 512), (512, 512)]
QTAIL = (1024, T - 1024)               # 5 queries -> batched-exp path
PROJ_CHUNKS = [(0, 343), (343, 343), (686, 343)]
NCHUNKS = [(0, 512), (512, 512)]


def build():
    nc = bacc.Bacc(None, target_bir_lowering=False)
    hs = nc.dram_tensor("hs", [H, TOK], bf16, kind="ExternalInput")  # host pre-transposed
    cos_d = nc.dram_tensor("cos", [PATCH, HD], f32, kind="ExternalInput")
    sin_d = nc.dram_tensor("sin", [PATCH, HD], f32, kind="ExternalInput")
    w_d = {wn: nc.dram_tensor(wn, [H, H], bf16, kind="ExternalInput")
           for wn in ("wq", "wk", "wv", "wo")}
    b_d = {"bq": nc.dram_tensor("bq", [H], f32, kind="ExternalInput"),
           "bv": nc.dram_tensor("bv", [H], bf16, kind="ExternalInput"),
           "bo": nc.dram_tensor("bo", [H], f32, kind="ExternalInput")}
    ident_d = nc.dram_tensor("ident", [128, 128], bf16, kind="ExternalInput")
    out_d = nc.dram_tensor("out", [H, TOK], f32, kind="ExternalOutput")

    with tile.TileContext(nc) as tc:
        with (
            tc.tile_pool(name="const", bufs=1) as cpool,
            tc.tile_pool(name="item", bufs=1) as ipool,
            tc.tile_pool(name="espool", bufs=9) as espool,
            tc.tile_pool(name="ntpool", bufs=8) as ntpool,
            tc.tile_pool(name="work", bufs=2) as wpool,
            tc.tile_pool(name="rope", bufs=2) as rpool,
            tc.tile_pool(name="attn", bufs=2) as apool,
            tc.tile_pool(name="ypool", bufs=2) as ypool,
            tc.tile_pool(name="ps_s", bufs=2, space="PSUM") as ps_s,
            tc.tile_pool(name="ps_av", bufs=1, space="PSUM") as ps_av,
            tc.tile_pool(name="ps_pt", bufs=1, space="PSUM") as ps_pt,
            tc.tile_pool(name="ps_w", bufs=2, space="PSUM") as ps_w,
        ):
            identb = cpool.tile([128, 128], bf16)
            nc.sync.dma_start(identb[:], ident_d[:])
            ident = cpool.tile([128, 128], f32)
            nc.vector.tensor_copy(ident[:], identb[:])

            # --- X-prep: hs is already feature-major; one strided DMA per item ---
            hs_r = hs.rearrange("(o p) t -> p o t", p=128)

            def emit_xprep_full(bi, XT):
                nc.sync.dma_start(XT[:, :, :], hs_r[:, :, bi * T: bi * T + T])

            XT0 = ipool.tile([128, KO, T], bf16, tag="XT", name="XT_0")
            emit_xprep_full(0, XT0)

            # --- cos/sin -> transposed, duplicated, sign-adjusted tables ---
            cosT2 = cpool.tile([128, PATCH], bf16)
            sinT2sw = cpool.tile([128, PATCH], bf16)
            cs_all = cpool.tile([128, PATCH // 128, HD], f32, tag="cs_all")
            sn_all = cpool.tile([128, PATCH // 128, HD], f32, tag="sn_all")
            nc.sync.dma_start(cs_all[:], cos_d.rearrange("(o p) d -> p o d", p=128))
            nc.sync.dma_start(sn_all[:], sin_d.rearrange("(o p) d -> p o d", p=128))
            for i in range(PATCH // 128):
                sl = slice(i * 128, (i + 1) * 128)
                pt = ps_w.tile([128, 512], f32, tag="ps_w")
                nc.tensor.transpose(pt[:HD, :128], cs_all[:, i, :], ident[:])
                nc.vector.tensor_copy(cosT2[0:64, sl], pt[0:64, :128])
                nc.vector.tensor_copy(cosT2[64:128, sl], pt[0:64, :128])
                pt2 = ps_w.tile([128, 512], f32, tag="ps_w")
                nc.tensor.transpose(pt2[:HD, :128], sn_all[:, i, :], ident[:])
                # rows 0:32 hold +sin[32:64] (read at source partitions 32:64 of
                # q), rows 32:64 hold -sin[0:32]; duplicated for the odd head.
                nc.vector.tensor_copy(sinT2sw[0:32, sl], pt2[32:64, :128])
                nc.vector.tensor_scalar_mul(sinT2sw[32:64, sl], pt2[0:32, :128],
                                            -1.0)
                nc.vector.tensor_copy(sinT2sw[64:96, sl], pt2[32:64, :128])
                nc.vector.tensor_scalar_mul(sinT2sw[96:128, sl], pt2[0:32, :128],
                                            -1.0)

            # --- biases, weights (already bf16; single strided DMA each) ---
            bq_sb = cpool.tile([128, KO], f32)
            nc.sync.dma_start(bq_sb[:], b_d["bq"].rearrange("(o p) -> p o", p=128))
            bo_sb = cpool.tile([128, KO], f32)
            nc.sync.dma_start(bo_sb[:], b_d["bo"].rearrange("(o p) -> p o", p=128))
            bv_bc = cpool.tile([128, H], bf16)
            nc.sync.dma_start(bv_bc[:], b_d["bv"][None, :].to_broadcast((128, H)))

            wb = {}
            for wn in ("wq", "wv", "wk", "wo"):
                wb[wn] = cpool.tile([128, KO, H], bf16, tag=f"wb_{wn}",
                                    name=f"wb_{wn}")
            for wn in ("wq", "wv", "wk", "wo"):
                nc.sync.dma_start(
                    wb[wn][:], w_d[wn].rearrange("(o p) n -> p o n", p=128))

            # ---------------- per batch item ----------------
            def make_item(bi, XT):
                tok0 = bi * T
                QT = ipool.tile([128, KO, T], bf16, tag="QT", name=f"QT_{bi}")
                KT = ipool.tile([128, KO, T], bf16, tag="KT", name=f"KT_{bi}")
                Vst = ipool.tile([128, NJT, NH, HD + 1], bf16, tag="Vst",
                                 name=f"Vst_{bi}")
                AOT = ipool.tile([128, KO, T], bf16, tag="AOT", name=f"AOT_{bi}")

                def emit_vinit():
                    nc.vector.memset(Vst[:, :, :, HD:HD + 1], 1.0)

                def emit_vproj_t(ci, ti):
                    n0, nw = NCHUNKS[ci]
                    t0, tw = TOK_TILES[ti]
                    pm = ps_w.tile([128, 512], f32, tag="ps_w",
                                   name=f"pmv_{bi}_{ci}_{ti}")
                    for ko in range(KO):
                        nc.tensor.matmul(
                            pm[:tw, :nw],
                            XT[:, ko, t0:t0 + tw],
                            wb["wv"][:, ko, n0:n0 + nw],
                            start=(ko == 0), stop=(ko == KO - 1))
                    nc.vector.tensor_tensor(
                        Vst[:tw, ti, n0 // HD:(n0 + nw) // HD, 0:HD],
                        pm[:tw, :nw], bv_bc[:tw, n0:n0 + nw], ADD)

                def emit_qkproj_g(mo, which, ci):
                    dst, wn, bias = ((QT, "wq", True), (KT, "wk", False))[which]
                    q0, qw = PROJ_CHUNKS[ci]
                    pm = ps_w.tile([128, 512], f32, tag="ps_w",
                                   name=f"pm_{bi}_{wn}_{mo}_{q0}")
                    for ko in range(KO):
                        nc.tensor.matmul(
                            pm[:, :qw],
                            wb[wn][:, ko, mo * 128:(mo + 1) * 128],
                            XT[:, ko, q0:q0 + qw],
                            start=(ko == 0), stop=(ko == KO - 1))
                    if bias:
                        nc.vector.tensor_scalar_add(
                            dst[:, mo, q0:q0 + qw], pm[:, :qw],
                            bq_sb[:, mo:mo + 1])
                    else:
                        nc.vector.tensor_copy(dst[:, mo, q0:q0 + qw], pm[:, :qw])

                def emit_rope_t(mo, which):
                    tgt = (QT, KT)[which]
                    src = tgt[:, mo, NPREF:T]
                    t1 = rpool.tile([128, PATCH], bf16, tag="rope1")
                    nc.vector.tensor_tensor(t1[:], src, cosT2[:], MUL)
                    t2 = rpool.tile([128, PATCH], bf16, tag="rope2")
                    for (o, sp) in ((0, 32), (32, 0), (64, 96), (96, 64)):
                        nc.vector.tensor_tensor(
                            t2[o:o + 32, :], tgt[sp:sp + 32, mo, NPREF:T],
                            sinT2sw[sp:sp + 32, :], MUL)
                    nc.vector.tensor_tensor(src, t1[:], t2[:], ADD)

                # --- attention, head h: phase A (S + exp into es ring) ---
                def emit_attn_a(h, es_tiles, pump=None):
                    ph = (h % 2) * 64
                    kq = h // 2
                    for ji, (j0, jw) in enumerate(TOK_TILES):
                        if pump is not None:
                            pump()
                        pss = ps_s.tile([128, 1024], f32, tag="ps_s")
                        for qi, (q0, qw) in enumerate(QCHUNKS):
                            nc.tensor.matmul(
                                pss[:jw, q0:q0 + qw],
                                KT[ph:ph + 64, kq, j0:j0 + jw],
                                QT[ph:ph + 64, kq, q0:q0 + qw],
                                start=True, stop=True)
                        es = espool.tile([128, 1024], bf16, tag="es",
                                         name=f"es_{bi}_{h}_{ji}")
                        nc.scalar.activation(es[:jw, :], pss[:jw, :],
                                             FP.Exp, scale=SCALE)
                        es_tiles[ji] = es

                # --- attention, head h: phase B (AV + normalize into nt) ---
                def emit_attn_b(h, es_tiles, nt_tiles, pump=None):
                    ph = (h % 2) * 64
                    for qc in range(2):
                        if pump is not None:
                            pump()
                        av = ps_av.tile([128, 4, HD + 1], f32, tag="av",
                                        name=f"av_{bi}_{h}_{qc}")
                        for qtl in range(4):
                            qt = qc * 4 + qtl
                            for ji, (j0, jw) in enumerate(TOK_TILES):
                                nc.tensor.matmul(
                                    av[:, qtl, :],
                                    es_tiles[ji][:jw, qt * 128:(qt + 1) * 128],
                                    Vst[:jw, ji, h, :],
                                    start=(ji == 0), stop=(ji == NJT - 1))
                        rc = apool.tile([128, 4], f32, tag="rc")
                        nc.vector.reciprocal(rc[:, :], av[:, :, HD])
                        for qtl in range(4):
                            qt = qc * 4 + qtl
                            nc.vector.tensor_scalar_mul(
                                nt_tiles[qt][:, ph:ph + HD],
                                av[:, qtl, 0:HD], rc[:, qtl:qtl + 1])

                # --- pair epilogue: transpose [q, d-pair] -> AOT feature-major ---
                def emit_pair_fin(kq, nt_tiles, pump=None):
                    pt = ps_pt.tile([128, 2, 128], bf16, tag="pt",
                                    name=f"pt_{bi}_{kq}")
                    for qt in range(0, NQT, 2):
                        if pump is not None:
                            pump()
                        nc.tensor.transpose(pt[:, 0, :], nt_tiles[qt][:, :],
                                            identb[:])
                        nc.tensor.transpose(pt[:, 1, :], nt_tiles[qt + 1][:, :],
                                            identb[:])
                        nc.vector.tensor_copy(
                            AOT[:, kq, qt * 128:(qt + 2) * 128], pt[:, :, :])

                def emit_tail():
                    # 5-query tail for all 16 heads, batched: S packed into one
                    # ps_s slot (heads 0..10 bank A, 11..15 bank B), two exps,
                    # AV accumulated per head into one ps_s slot.
                    qt0, qtw = QTAIL
                    pst = ps_s.tile([128, 1024], f32, tag="ps_s",
                                    name=f"pst_{bi}")
                    nc.vector.memset(pst[:], 0.0)

                    def tcol(h):
                        return (h * qtw * NJT if h <= 10
                                else 512 + (h - 11) * qtw * NJT)

                    for h in range(NH):
                        ph = (h % 2) * 64
                        kq = h // 2
                        for ji, (j0, jw) in enumerate(TOK_TILES):
                            nc.tensor.matmul(
                                pst[:jw,
                                    tcol(h) + ji * qtw: tcol(h) + (ji + 1) * qtw],
                                KT[ph:ph + 64, kq, j0:j0 + jw],
                                QT[ph:ph + 64, kq, qt0:qt0 + qtw],
                                start=True, stop=True)
                    est = apool.tile([128, 1024], bf16, tag="expS",
                                     name=f"est_{bi}")
                    nc.scalar.activation(est[:, 0:495], pst[:, 0:495],
                                         FP.Exp, scale=SCALE)
                    nc.scalar.activation(est[:, 512:737], pst[:, 512:737],
                                         FP.Exp, scale=SCALE)
                    pot = ps_s.tile([128, 1024], f32, tag="ps_s",
                                    name=f"pot_{bi}")
                    for h in range(NH):
                        for ji, (j0, jw) in enumerate(TOK_TILES):
                            nc.tensor.matmul(
                                pot[:HD + 1, h * qtw:(h + 1) * qtw],
                                Vst[:jw, ji, h, :],
                                est[0:jw,
                                    tcol(h) + ji * qtw: tcol(h) + (ji + 1) * qtw],
                                start=(ji == 0), stop=(ji == NJT - 1))
                    rc = apool.tile([1, 512], f32, tag="recip")
                    nc.vector.reciprocal(rc[0:1, :NH * qtw],
                                         pot[64:65, :NH * qtw])
                    rb = apool.tile([64, 512], f32, tag="recipB")
                    nc.gpsimd.partition_broadcast(rb[:, :NH * qtw],
                                                  rc[0:1, :NH * qtw])
                    for h in range(NH):
                        nc.vector.tensor_tensor(
                            AOT[(h % 2) * 64:(h % 2) * 64 + 64, h // 2,
                                qt0:qt0 + qtw],
                            pot[0:64, h * qtw:(h + 1) * qtw],
                            rb[:, h * qtw:(h + 1) * qtw], MUL)

                # --- out-projection, transposed: yT[mo-block, tokens] ---
                def emit_outproj_g(mo, ci):
                    t0, tw = PROJ_CHUNKS[ci]
                    pm = ps_w.tile([128, 512], f32, tag="ps_w",
                                   name=f"pmo_{bi}_{mo}_{t0}")
                    for ko in range(KO):
                        nc.tensor.matmul(
                            pm[:, :tw],
                            wb["wo"][:, ko, mo * 128:(mo + 1) * 128],
                            AOT[:, ko, t0:t0 + tw],
                            start=(ko == 0), stop=(ko == KO - 1))
                    y = ypool.tile([128, 512], f32, tag="y")
                    nc.vector.tensor_scalar_add(y[:, :tw], pm[:, :tw],
                                                bo_sb[:, mo:mo + 1])
                    nc.sync.dma_start(
                        out_d[mo * 128:(mo + 1) * 128, tok0 + t0: tok0 + t0 + tw],
                        y[:, :tw])

                def emit_outproj(skip=()):
                    for mo in range(KO):
                        for ci in range(len(PROJ_CHUNKS)):
                            if (mo, ci) not in skip:
                                emit_outproj_g(mo, ci)

                def emit_blocks(extra=None):
                    fills = []

                    def pump():
                        if fills:
                            fills.pop(0)()

                    for kq in range(KO):
                        if kq == 3:
                            fills.extend(
                                (lambda ti=ti: emit_vproj_t(1, ti))
                                for ti in range(NJT))
                        if kq < KO - 1:
                            fills.extend(
                                (lambda kq=kq, which=which, ci=ci:
                                 emit_qkproj_g(kq + 1, which, ci))
                                for which in range(2)
                                for ci in range(len(PROJ_CHUNKS)))
                            fills.append(lambda kq=kq: emit_rope_t(kq + 1, 0))
                            fills.append(lambda kq=kq: emit_rope_t(kq + 1, 1))
                        if extra and kq in extra:
                            fills.extend(extra[kq])
                        nt_tiles = [
                            ntpool.tile([128, 128], bf16, tag="nt",
                                        name=f"nt_{bi}_{kq}_{qt}")
                            for qt in range(NQT)]
                        for h in (2 * kq, 2 * kq + 1):
                            es_tiles = [None] * NJT
                            emit_attn_a(h, es_tiles, pump)
                            emit_attn_b(h, es_tiles, nt_tiles, pump)
                        emit_pair_fin(kq, nt_tiles, pump)
                    while fills:
                        fills.pop(0)()

                def emit_head():
                    emit_vinit()
                    for ti in range(NJT):
                        emit_vproj_t(0, ti)
                    for which in range(2):
                        for ci in range(len(PROJ_CHUNKS)):
                            emit_qkproj_g(0, which, ci)
                    emit_rope_t(0, 0)
                    emit_rope_t(0, 1)

                return {
                    "head": emit_head, "blocks": emit_blocks,
                    "tail": emit_tail, "outproj": emit_outproj,
                    "outproj_g": emit_outproj_g,
                }

            it0 = make_item(0, XT0)
            it0["head"]()
            XT1 = ipool.tile([128, KO, T], bf16, tag="XT", name="XT_1")
            it0["blocks"](extra={7: [lambda: emit_xprep_full(1, XT1)]})
            it0["tail"]()
            it1 = make_item(1, XT1)
            it1["head"]()              # runs during item0 out-proj window
            # defer all of item0's out-proj into item1's blocks as pump fills
            defer = [(mo, ci) for mo in range(KO)
                     for ci in range(len(PROJ_CHUNKS))]
            dthunks = [(lambda mo=mo, ci=ci: it0["outproj_g"](mo, ci))
                       for (mo, ci) in defer]
            it1["blocks"](extra={kq: dthunks[3 * kq:3 * kq + 3]
                                 for kq in range(KO)})
            it1["tail"]()
            it1["outproj"]()

    nc.compile()
    return nc


_NC_CACHE = []
_LAST_RESULT = []


def kernel(hidden_states, cos, sin, wq, bq, wk, wv, bv, wo, bo):
    from concourse.bass_utils import run_bass_kernel_spmd

    def _bf16(x):
        return np.ascontiguousarray(np.asarray(x).astype(ml_dtypes.bfloat16))

    def _f32(x):
        return np.ascontiguousarray(np.asarray(x, dtype=np.float32))

    hs_b = _bf16(hidden_states).reshape(B * T, H)
    shared = {
        "ident": np.eye(128, dtype=ml_dtypes.bfloat16),
        "cos": _f32(cos), "sin": _f32(sin),
        "wq": _bf16(wq), "wk": _bf16(wk), "wv": _bf16(wv), "wo": _bf16(wo),
        "bq": _f32(bq), "bv": _bf16(bv), "bo": _f32(bo),
    }
    if not _NC_CACHE:
        _NC_CACHE.append(build())
    nc = _NC_CACHE[0]

    in_maps = []
    for c in range(NCORES):
        m = dict(shared)
        m["hs"] = np.ascontiguousarray(hs_b[c * TOK:(c + 1) * TOK].T)
        in_maps.append(m)

    try:
        res = run_bass_kernel_spmd(nc, in_maps, core_ids=list(range(NCORES)))
    except Exception:
        # transient NRT device errors (e.g. NRT_EXEC_UNIT_UNRECOVERABLE) have
        # been observed on this fabric; one retry usually succeeds
        time.sleep(2.0)
        res = run_bass_kernel_spmd(nc, in_maps, core_ids=list(range(NCORES)))
    _LAST_RESULT.clear()
    _LAST_RESULT.append(res)
    out = np.concatenate(
        [r["out"].T.reshape(BPC, T, H) for r in res.results], axis=0)
    return out


# revision 6
# speedup vs baseline: 1.0454x; 1.0454x over previous
"""Dinov3 ViT attention kernel for Trainium2 (8 NeuronCores, data-parallel over batch).

Per core: 2 batch items. hidden_states [2*1029, 1024] in, out [2*1029, 1024] f32.

Host pre-casts hidden_states + weights to bf16 (the kernel computes in bf16
internally anyway, so this only halves DMA traffic).

Per item pipeline (PE-dense, interleaved with ACT-bound attention):
  X-prep (strided DMA to feature-major XT) ->
  V-proj chunk 0 (heads 0..7) ->
  for kq in 0..7:  # head pair (2kq, 2kq+1)
    per head: phase A: S^T per key-tile (K=64 matmul) -> exp on ScalarE
              (scale=1/8, no max: |scores| < ~7) into a 9-slot SBUF ring;
              phase B: AV with es as the STATIONARY operand -> out
              [128 queries, 65] per query tile (col 64 = softmax sum via
              ones-augmented V), accumulated over key tiles in one PSUM bank;
              normalize via per-partition reciprocal + tensor_scalar.
    per pair: PE-transpose normalized [q, d] tiles back to feature-major
              AOT, PSUM->SBUF copy on the (otherwise idle) Pool engine.
    (Q/K proj mo+1, RoPE, V chunk 1, prev-item out-proj pumped into gaps.)
  5-query tail batched into one [128,45] PSUM bank + single exp per bank,
  normalized via DVE reciprocal + gpsimd partition_broadcast.
  out-projection emitted transposed: Y^T = Wo^T-stationary @ AOT, bias via
  per-partition tensor_scalar, DMA'd as [H, TOK] f32 (host transposes back).
"""
import sys
import time

sys.path.insert(0, "/opt/trn_rl_repo")

import ml_dtypes
import numpy as np

import concourse.bacc as bacc
import concourse.mybir as mybir
import concourse.tile as tile

f32 = mybir.dt.float32
bf16 = mybir.dt.bfloat16
FP = mybir.ActivationFunctionType
ADD = mybir.AluOpType.add
MUL = mybir.AluOpType.mult

H = 1024
NH = 16
HD = 64
T = 1029
NPREF = 5
PATCH = 1024
B = 16
NCORES = 8
BPC = B // NCORES          # batch items per core
KO = H // 128              # 8 feature k-tiles
TOK = BPC * T              # tokens per core (2058)
SCALE = 1.0 / float(np.sqrt(HD))

TOK_TILES = [(i * 128, min(128, T - i * 128)) for i in range((T + 127) // 128)]
NJT = len(TOK_TILES)                   # 9 key tiles (8 full + 5)
NQT = 8                                # full 128-query tiles (0..1024)
QCHUNKS = [(0, 512), (512, 512)]
QTAIL = (1024, T - 1024)               # 5 queries -> batched-exp path
PROJ_CHUNKS = [(0, 343), (343, 343), (686, 343)]
NCHUNKS = [(0, 512), (512, 512)]


def build():
    nc = bacc.Bacc(None, target_bir_lowering=False)
    hs = nc.dram_tensor("hs", [H, TOK], bf16, kind="ExternalInput")  # host pre-transposed
    cos_d = nc.dram_tensor("cos", [PATCH, HD], f32, kind="ExternalInput")
    sin_d = nc.dram_tensor("sin", [PATCH, HD], f32, kind="ExternalInput")
    w_d = {wn: nc.dram_tensor(wn, [H, H], bf16, kind="ExternalInput")
           for wn in ("wq", "wk", "wv", "wo")}
    b_d = {"bq": nc.dram_tensor("bq", [H], f32, kind="ExternalInput"),
           "bv": nc.dram_tensor("bv", [H], bf16, kind="ExternalInput"),
           "bo": nc.dram_tensor("bo", [H], f32, kind="ExternalInput")}
    ident_d = nc.dram_tensor("ident", [128, 128], bf16, kind="ExternalInput")
    out_d = nc.dram_tensor("out", [H, TOK], f32, kind="ExternalOutput")

    with tile.TileContext(nc) as tc:
        with (
            tc.tile_pool(name="const", bufs=1) as cpool,
            tc.tile_pool(name="item", bufs=1) as ipool,
            tc.tile_pool(name="espool", bufs=9) as espool,
            tc.tile_pool(name="ntpool", bufs=8) as ntpool,
            tc.tile_pool(name="work", bufs=2) as wpool,
            tc.tile_pool(name="rope", bufs=2) as rpool,
            tc.tile_pool(name="attn", bufs=2) as apool,
            tc.tile_pool(name="ypool", bufs=2) as ypool,
            tc.tile_pool(name="ps_s", bufs=2, space="PSUM") as ps_s,
            tc.tile_pool(name="ps_av", bufs=1, space="PSUM") as ps_av,
            tc.tile_pool(name="ps_pt", bufs=1, space="PSUM") as ps_pt,
            tc.tile_pool(name="ps_w", bufs=2, space="PSUM") as ps_w,
        ):
            identb = cpool.tile([128, 128], bf16)
            nc.sync.dma_start(identb[:], ident_d[:])
            ident = cpool.tile([128, 128], f32)
            nc.vector.tensor_copy(ident[:], identb[:])

            # --- X-prep: hs is already feature-major; one strided DMA per item ---
            hs_r = hs.rearrange("(o p) t -> p o t", p=128)

            def emit_xprep_full(bi, XT):
                nc.sync.dma_start(XT[:, :, :], hs_r[:, :, bi * T: bi * T + T])

            XT0 = ipool.tile([128, KO, T], bf16, tag="XT", name="XT_0")
            emit_xprep_full(0, XT0)

            # --- cos/sin -> transposed, duplicated, sign-adjusted tables ---
            cosT2 = cpool.tile([128, PATCH], bf16)
            sinT2sw = cpool.tile([128, PATCH], bf16)
            cs_all = cpool.tile([128, PATCH // 128, HD], f32, tag="cs_all")
            sn_all = cpool.tile([128, PATCH // 128, HD], f32, tag="sn_all")
            nc.sync.dma_start(cs_all[:], cos_d.rearrange("(o p) d -> p o d", p=128))
            nc.sync.dma_start(sn_all[:], sin_d.rearrange("(o p) d -> p o d", p=128))
            for i in range(PATCH // 128):
                sl = slice(i * 128, (i + 1) * 128)
                pt = ps_w.tile([128, 512], f32, tag="ps_w")
                nc.tensor.transpose(pt[:HD, :128], cs_all[:, i, :], ident[:])
                nc.vector.tensor_copy(cosT2[0:64, sl], pt[0:64, :128])
                nc.vector.tensor_copy(cosT2[64:128, sl], pt[0:64, :128])
                pt2 = ps_w.tile([128, 512], f32, tag="ps_w")
                nc.tensor.transpose(pt2[:HD, :128], sn_all[:, i, :], ident[:])
                # rows 0:32 hold +sin[32:64] (read at source partitions 32:64 of
                # q), rows 32:64 hold -sin[0:32]; duplicated for the odd head.
                nc.vector.tensor_copy(sinT2sw[0:32, sl], pt2[32:64, :128])
                nc.vector.tensor_scalar_mul(sinT2sw[32:64, sl], pt2[0:32, :128],
                                            -1.0)
                nc.vector.tensor_copy(sinT2sw[64:96, sl], pt2[32:64, :128])
                nc.vector.tensor_scalar_mul(sinT2sw[96:128, sl], pt2[0:32, :128],
                                            -1.0)

            # --- biases, weights (already bf16; single strided DMA each) ---
            bq_sb = cpool.tile([128, KO], f32)
            nc.sync.dma_start(bq_sb[:], b_d["bq"].rearrange("(o p) -> p o", p=128))
            bo_sb = cpool.tile([128, KO], f32)
            nc.sync.dma_start(bo_sb[:], b_d["bo"].rearrange("(o p) -> p o", p=128))
            bv_bc = cpool.tile([128, H], bf16)
            nc.sync.dma_start(bv_bc[:], b_d["bv"][None, :].to_broadcast((128, H)))

            wb = {}
            for wn in ("wq", "wv", "wk", "wo"):
                wb[wn] = cpool.tile([128, KO, H], bf16, tag=f"wb_{wn}",
                                    name=f"wb_{wn}")
            for wn in ("wq", "wv", "wk", "wo"):
                nc.sync.dma_start(
                    wb[wn][:], w_d[wn].rearrange("(o p) n -> p o n", p=128))

            # ---------------- per batch item ----------------
            def make_item(bi, XT):
                tok0 = bi * T
                QT = ipool.tile([128, KO, T], bf16, tag="QT", name=f"QT_{bi}")
                KT = ipool.tile([128, KO, T], bf16, tag="KT", name=f"KT_{bi}")
                Vst = ipool.tile([128, NJT, NH, HD + 1], bf16, tag="Vst",
                                 name=f"Vst_{bi}")
                AOT = ipool.tile([128, KO, T], bf16, tag="AOT", name=f"AOT_{bi}")

                def emit_vinit():
                    nc.vector.memset(Vst[:, :, :, HD:HD + 1], 1.0)

                def emit_vproj_t(ci, ti):
                    n0, nw = NCHUNKS[ci]
                    t0, tw = TOK_TILES[ti]
                    pm = ps_w.tile([128, 512], f32, tag="ps_w",
                                   name=f"pmv_{bi}_{ci}_{ti}")
                    for ko in range(KO):
                        nc.tensor.matmul(
                            pm[:tw, :nw],
                            XT[:, ko, t0:t0 + tw],
                            wb["wv"][:, ko, n0:n0 + nw],
                            start=(ko == 0), stop=(ko == KO - 1))
                    nc.vector.tensor_tensor(
                        Vst[:tw, ti, n0 // HD:(n0 + nw) // HD, 0:HD],
                        pm[:tw, :nw], bv_bc[:tw, n0:n0 + nw], ADD)

                def emit_qkproj_g(mo, which, ci):
                    dst, wn, bias = ((QT, "wq", True), (KT, "wk", False))[which]
                    q0, qw = PROJ_CHUNKS[ci]
                    pm = ps_w.tile([128, 512], f32, tag="ps_w",
                                   name=f"pm_{bi}_{wn}_{mo}_{q0}")
                    for ko in range(KO):
                        nc.tensor.matmul(
                            pm[:, :qw],
                            wb[wn][:, ko, mo * 128:(mo + 1) * 128],
                            XT[:, ko, q0:q0 + qw],
                            start=(ko == 0), stop=(ko == KO - 1))
                    if bias:
                        nc.vector.tensor_scalar_add(
                            dst[:, mo, q0:q0 + qw], pm[:, :qw],
                            bq_sb[:, mo:mo + 1])
                    else:
                        nc.vector.tensor_copy(dst[:, mo, q0:q0 + qw], pm[:, :qw])

                def emit_rope_t(mo, which):
                    tgt = (QT, KT)[which]
                    src = tgt[:, mo, NPREF:T]
                    t1 = rpool.tile([128, PATCH], bf16, tag="rope1")
                    nc.vector.tensor_tensor(t1[:], src, cosT2[:], MUL)
                    t2 = rpool.tile([128, PATCH], bf16, tag="rope2")
                    for (o, sp) in ((0, 32), (32, 0), (64, 96), (96, 64)):
                        nc.vector.tensor_tensor(
                            t2[o:o + 32, :], tgt[sp:sp + 32, mo, NPREF:T],
                            sinT2sw[sp:sp + 32, :], MUL)
                    nc.vector.tensor_tensor(src, t1[:], t2[:], ADD)

                # --- attention, head h: phase A (S + exp into es ring) ---
                def emit_attn_a(h, es_tiles, pump=None):
                    ph = (h % 2) * 64
                    kq = h // 2
                    for ji, (j0, jw) in enumerate(TOK_TILES):
                        if pump is not None:
                            pump()
                        pss = ps_s.tile([128, 1024], f32, tag="ps_s")
                        for qi, (q0, qw) in enumerate(QCHUNKS):
                            nc.tensor.matmul(
                                pss[:jw, q0:q0 + qw],
                                KT[ph:ph + 64, kq, j0:j0 + jw],
                                QT[ph:ph + 64, kq, q0:q0 + qw],
                                start=True, stop=True)
                        es = espool.tile([128, 1024], bf16, tag="es",
                                         name=f"es_{bi}_{h}_{ji}")
                        nc.scalar.activation(es[:jw, :], pss[:jw, :],
                                             FP.Exp, scale=SCALE)
                        es_tiles[ji] = es

                # --- attention, head h: phase B (AV + normalize into nt) ---
                def emit_attn_b(h, es_tiles, nt_tiles, pump=None):
                    ph = (h % 2) * 64
                    for qc in range(2):
                        if pump is not None:
                            pump()
                        av = ps_av.tile([128, 4, HD + 1], f32, tag="av",
                                        name=f"av_{bi}_{h}_{qc}")
                        # ji-major so the PE stream only ever waits on the
                        # most recent exp, not the last one
                        for ji, (j0, jw) in enumerate(TOK_TILES):
                            if pump is not None and ji % 3 == 2:
                                pump()
                            for qtl in range(4):
                                qt = qc * 4 + qtl
                                nc.tensor.matmul(
                                    av[:, qtl, :],
                                    es_tiles[ji][:jw, qt * 128:(qt + 1) * 128],
                                    Vst[:jw, ji, h, :],
                                    start=(ji == 0), stop=(ji == NJT - 1))
                        rc = apool.tile([128, 4], f32, tag="rc")
                        nc.vector.reciprocal(rc[:, :], av[:, :, HD])
                        for qtl in range(4):
                            qt = qc * 4 + qtl
                            nc.vector.tensor_scalar_mul(
                                nt_tiles[qt][:, ph:ph + HD],
                                av[:, qtl, 0:HD], rc[:, qtl:qtl + 1])

                # --- pair epilogue: transpose [q, d-pair] -> AOT feature-major ---
                def emit_pair_fin(kq, nt_tiles, pump=None):
                    pt = ps_pt.tile([128, 2, 128], bf16, tag="pt",
                                    name=f"pt_{bi}_{kq}")
                    for qt in range(0, NQT, 2):
                        if pump is not None:
                            pump()
                        nc.tensor.transpose(pt[:, 0, :], nt_tiles[qt][:, :],
                                            identb[:])
                        nc.tensor.transpose(pt[:, 1, :], nt_tiles[qt + 1][:, :],
                                            identb[:])
                        nc.vector.tensor_copy(
                            AOT[:, kq, qt * 128:(qt + 2) * 128], pt[:, :, :])

                def emit_tail():
                    # 5-query tail for all 16 heads, batched: S packed into one
                    # ps_s slot (heads 0..10 bank A, 11..15 bank B), two exps,
                    # AV accumulated per head into one ps_s slot.
                    qt0, qtw = QTAIL
                    pst = ps_s.tile([128, 1024], f32, tag="ps_s",
                                    name=f"pst_{bi}")
                    nc.vector.memset(pst[:], 0.0)

                    def tcol(h):
                        return (h * qtw * NJT if h <= 10
                                else 512 + (h - 11) * qtw * NJT)

                    for h in range(NH):
                        ph = (h % 2) * 64
                        kq = h // 2
                        for ji, (j0, jw) in enumerate(TOK_TILES):
                            nc.tensor.matmul(
                                pst[:jw,
                                    tcol(h) + ji * qtw: tcol(h) + (ji + 1) * qtw],
                                KT[ph:ph + 64, kq, j0:j0 + jw],
                                QT[ph:ph + 64, kq, qt0:qt0 + qtw],
                                start=True, stop=True)
                    est = apool.tile([128, 1024], bf16, tag="expS",
                                     name=f"est_{bi}")
                    nc.scalar.activation(est[:, 0:495], pst[:, 0:495],
                                         FP.Exp, scale=SCALE)
                    nc.scalar.activation(est[:, 512:737], pst[:, 512:737],
                                         FP.Exp, scale=SCALE)
                    pot = ps_s.tile([128, 1024], f32, tag="ps_s",
                                    name=f"pot_{bi}")
                    for h in range(NH):
                        for ji, (j0, jw) in enumerate(TOK_TILES):
                            nc.tensor.matmul(
                                pot[:HD + 1, h * qtw:(h + 1) * qtw],
                                Vst[:jw, ji, h, :],
                                est[0:jw,
                                    tcol(h) + ji * qtw: tcol(h) + (ji + 1) * qtw],
                                start=(ji == 0), stop=(ji == NJT - 1))
                    rc = apool.tile([1, 512], f32, tag="recip")
                    nc.vector.reciprocal(rc[0:1, :NH * qtw],
                                         pot[64:65, :NH * qtw])
                    rb = apool.tile([64, 512], f32, tag="recipB")
                    nc.gpsimd.partition_broadcast(rb[:, :NH * qtw],
                                                  rc[0:1, :NH * qtw])
                    for h in range(NH):
                        nc.vector.tensor_tensor(
                            AOT[(h % 2) * 64:(h % 2) * 64 + 64, h // 2,
                                qt0:qt0 + qtw],
                            pot[0:64, h * qtw:(h + 1) * qtw],
                            rb[:, h * qtw:(h + 1) * qtw], MUL)

                # --- out-projection, transposed: yT[mo-block, tokens] ---
                def emit_outproj_g(mo, ci):
                    t0, tw = PROJ_CHUNKS[ci]
                    pm = ps_w.tile([128, 512], f32, tag="ps_w",
                                   name=f"pmo_{bi}_{mo}_{t0}")
                    for ko in range(KO):
                        nc.tensor.matmul(
                            pm[:, :tw],
                            wb["wo"][:, ko, mo * 128:(mo + 1) * 128],
                            AOT[:, ko, t0:t0 + tw],
                            start=(ko == 0), stop=(ko == KO - 1))
                    y = ypool.tile([128, 512], f32, tag="y")
                    nc.vector.tensor_scalar_add(y[:, :tw], pm[:, :tw],
                                                bo_sb[:, mo:mo + 1])
                    nc.sync.dma_start(
                        out_d[mo * 128:(mo + 1) * 128, tok0 + t0: tok0 + t0 + tw],
                        y[:, :tw])

                def emit_outproj(skip=()):
                    for mo in range(KO):
                        for ci in range(len(PROJ_CHUNKS)):
                            if (mo, ci) not in skip:
                                emit_outproj_g(mo, ci)

                def emit_blocks(extra=None):
                    fills = []

                    def pump():
                        if fills:
                            fills.pop(0)()

                    for kq in range(KO):
                        if kq == 3:
                            fills.extend(
                                (lambda ti=ti: emit_vproj_t(1, ti))
                                for ti in range(NJT))
                        if kq < KO - 1:
                            fills.extend(
                                (lambda kq=kq, which=which, ci=ci:
                                 emit_qkproj_g(kq + 1, which, ci))
                                for which in range(2)
                                for ci in range(len(PROJ_CHUNKS)))
                            fills.append(lambda kq=kq: emit_rope_t(kq + 1, 0))
                            fills.append(lambda kq=kq: emit_rope_t(kq + 1, 1))
                        if extra and kq in extra:
                            fills.extend(extra[kq])
                        nt_tiles = [
                            ntpool.tile([128, 128], bf16, tag="nt",
                                        name=f"nt_{bi}_{kq}_{qt}")
                            for qt in range(NQT)]
                        for h in (2 * kq, 2 * kq + 1):
                            es_tiles = [None] * NJT
                            emit_attn_a(h, es_tiles, pump)
                            emit_attn_b(h, es_tiles, nt_tiles, pump)
                        emit_pair_fin(kq, nt_tiles, pump)
                    while fills:
                        fills.pop(0)()

                def emit_head():
                    emit_vinit()
                    for ti in range(NJT):
                        emit_vproj_t(0, ti)
                    for which in range(2):
                        for ci in range(len(PROJ_CHUNKS)):
                            emit_qkproj_g(0, which, ci)
                    emit_rope_t(0, 0)
                    emit_rope_t(0, 1)

                return {
                    "head": emit_head, "blocks": emit_blocks,
                    "tail": emit_tail, "outproj": emit_outproj,
                    "outproj_g": emit_outproj_g,
                }

            it0 = make_item(0, XT0)
            it0["head"]()
            XT1 = ipool.tile([128, KO, T], bf16, tag="XT", name="XT_1")
            it0["blocks"](extra={7: [lambda: emit_xprep_full(1, XT1)]})
            it0["tail"]()
            it1 = make_item(1, XT1)
            it1["head"]()              # runs during item0 out-proj window
            # defer all of item0's out-proj into item1's blocks as pump fills
            defer = [(mo, ci) for mo in range(KO)
                     for ci in range(len(PROJ_CHUNKS))]
            dthunks = [(lambda mo=mo, ci=ci: it0["outproj_g"](mo, ci))
                       for (mo, ci) in defer]
            it1["blocks"](extra={kq: dthunks[3 * kq:3 * kq + 3]
                                 for kq in range(KO)})
            it1["tail"]()
            it1["outproj"]()

    nc.compile()
    return nc


_NC_CACHE = []
_LAST_RESULT = []


def kernel(hidden_states, cos, sin, wq, bq, wk, wv, bv, wo, bo):
    from concourse.bass_utils import run_bass_kernel_spmd

    def _bf16(x):
        return np.ascontiguousarray(np.asarray(x).astype(ml_dtypes.bfloat16))

    def _f32(x):
        return np.ascontiguousarray(np.asarray(x, dtype=np.float32))

    hs_b = _bf16(hidden_states).reshape(B * T, H)
    shared = {
        "ident": np.eye(128, dtype=ml_dtypes.bfloat16),
        "cos": _f32(cos), "sin": _f32(sin),
        "wq": _bf16(wq), "wk": _bf16(wk), "wv": _bf16(wv), "wo": _bf16(wo),
        "bq": _f32(bq), "bv": _bf16(bv), "bo": _f32(bo),
    }
    if not _NC_CACHE:
        _NC_CACHE.append(build())
    nc = _NC_CACHE[0]

    in_maps = []
    for c in range(NCORES):
        m = dict(shared)
        m["hs"] = np.ascontiguousarray(hs_b[c * TOK:(c + 1) * TOK].T)
        in_maps.append(m)

    try:
        res = run_bass_kernel_spmd(nc, in_maps, core_ids=list(range(NCORES)))
    except Exception:
        # transient NRT device errors (e.g. NRT_EXEC_UNIT_UNRECOVERABLE) have
        # been observed on this fabric; one retry usually succeeds
        time.sleep(2.0)
        res = run_bass_kernel_spmd(nc, in_maps, core_ids=list(range(NCORES)))
    _LAST_RESULT.clear()
    _LAST_RESULT.append(res)
    out = np.concatenate(
        [r["out"].T.reshape(BPC, T, H) for r in res.results], axis=0)
    return out


# revision 11
# speedup vs baseline: 1.0463x; 1.0009x over previous
"""Dinov3 ViT attention kernel for Trainium2 (8 NeuronCores, data-parallel over batch).

Per core: 2 batch items. hidden_states [2*1029, 1024] in, out [2*1029, 1024] f32.

Host pre-casts hidden_states + weights to bf16 (the kernel computes in bf16
internally anyway, so this only halves DMA traffic).

Per item pipeline (PE-dense, interleaved with ACT-bound attention):
  X-prep (strided DMA to feature-major XT) ->
  V-proj chunk 0 (heads 0..7) ->
  for kq in 0..7:  # head pair (2kq, 2kq+1)
    per head: phase A: S^T per key-tile (K=64 matmul) -> exp on ScalarE
              (scale=1/8, no max: |scores| < ~7) into a 9-slot SBUF ring;
              phase B: AV with es as the STATIONARY operand -> out
              [128 queries, 65] per query tile (col 64 = softmax sum via
              ones-augmented V), accumulated over key tiles in one PSUM bank;
              normalize via per-partition reciprocal + tensor_scalar.
    per pair: PE-transpose normalized [q, d] tiles back to feature-major
              AOT, PSUM->SBUF copy on the (otherwise idle) Pool engine.
    (Q/K proj mo+1, RoPE, V chunk 1, prev-item out-proj pumped into gaps.)
  5-query tail batched into one [128,45] PSUM bank + single exp per bank,
  normalized via DVE reciprocal + gpsimd partition_broadcast.
  out-projection emitted transposed: Y^T = Wo^T-stationary @ AOT, bias via
  per-partition tensor_scalar, DMA'd as [H, TOK] f32 (host transposes back).
"""
import sys
import time

sys.path.insert(0, "/opt/trn_rl_repo")

import ml_dtypes
import numpy as np

import concourse.bacc as bacc
import concourse.mybir as mybir
import concourse.tile as tile

f32 = mybir.dt.float32
bf16 = mybir.dt.bfloat16
FP = mybir.ActivationFunctionType
ADD = mybir.AluOpType.add
MUL = mybir.AluOpType.mult

H = 1024
NH = 16
HD = 64
T = 1029
NPREF = 5
PATCH = 1024
B = 16
NCORES = 8
BPC = B // NCORES          # batch items per core
KO = H // 128              # 8 feature k-tiles
TOK = BPC * T              # tokens per core (2058)
SCALE = 1.0 / float(np.sqrt(HD))

TOK_TILES = [(i * 128, min(128, T - i * 128)) for i in range((T + 127) // 128)]
NJT = len(TOK_TILES)                   # 9 key tiles (8 full + 5)
NQT = 8                                # full 128-query tiles (0..1024)
QCHUNKS = [(0, 512), (512, 512)]
QTAIL = (1024, T - 1024)               # 5 queries -> batched-exp path
PROJ_CHUNKS = [(0, 343), (343, 343), (686, 343)]
NCHUNKS = [(0, 512), (512, 512)]


def build():
    nc = bacc.Bacc(None, target_bir_lowering=False)
    hs = nc.dram_tensor("hs", [H, TOK], bf16, kind="ExternalInput")  # host pre-transposed
    cos_d = nc.dram_tensor("cos", [PATCH, HD], f32, kind="ExternalInput")
    sin_d = nc.dram_tensor("sin", [PATCH, HD], f32, kind="ExternalInput")
    w_d = {wn: nc.dram_tensor(wn, [H, H], bf16, kind="ExternalInput")
           for wn in ("wq", "wk", "wv", "wo")}
    b_d = {"bq": nc.dram_tensor("bq", [H], f32, kind="ExternalInput"),
           "bv": nc.dram_tensor("bv", [H], bf16, kind="ExternalInput"),
           "bo": nc.dram_tensor("bo", [H], f32, kind="ExternalInput")}
    ident_d = nc.dram_tensor("ident", [128, 128], bf16, kind="ExternalInput")
    out_d = nc.dram_tensor("out", [H, TOK], f32, kind="ExternalOutput")

    with tile.TileContext(nc) as tc:
        with (
            tc.tile_pool(name="const", bufs=1) as cpool,
            tc.tile_pool(name="item", bufs=1) as ipool,
            tc.tile_pool(name="espool", bufs=9) as espool,
            tc.tile_pool(name="ntpool", bufs=8) as ntpool,
            tc.tile_pool(name="work", bufs=2) as wpool,
            tc.tile_pool(name="rope", bufs=2) as rpool,
            tc.tile_pool(name="attn", bufs=2) as apool,
            tc.tile_pool(name="ypool", bufs=2) as ypool,
            tc.tile_pool(name="ps_s", bufs=2, space="PSUM") as ps_s,
            tc.tile_pool(name="ps_av", bufs=1, space="PSUM") as ps_av,
            tc.tile_pool(name="ps_pt", bufs=1, space="PSUM") as ps_pt,
            tc.tile_pool(name="ps_w", bufs=2, space="PSUM") as ps_w,
        ):
            identb = cpool.tile([128, 128], bf16)
            nc.sync.dma_start(identb[:], ident_d[:])
            ident = cpool.tile([128, 128], f32)
            nc.vector.tensor_copy(ident[:], identb[:])

            # --- X-prep: hs is already feature-major; one strided DMA per item ---
            hs_r = hs.rearrange("(o p) t -> p o t", p=128)

            def emit_xprep_full(bi, XT, nsplit=3):
                step = (T + nsplit - 1) // nsplit
                for t0 in range(0, T, step):
                    tw = min(step, T - t0)
                    nc.sync.dma_start(
                        XT[:, :, t0:t0 + tw],
                        hs_r[:, :, bi * T + t0: bi * T + t0 + tw])

            XT0 = ipool.tile([128, KO, T], bf16, tag="XT", name="XT_0")
            emit_xprep_full(0, XT0)

            # --- cos/sin -> transposed, duplicated, sign-adjusted tables ---
            cosT2 = cpool.tile([128, PATCH], bf16)
            sinT2sw = cpool.tile([128, PATCH], bf16)
            cs_all = cpool.tile([128, PATCH // 128, HD], f32, tag="cs_all")
            sn_all = cpool.tile([128, PATCH // 128, HD], f32, tag="sn_all")
            nc.sync.dma_start(cs_all[:], cos_d.rearrange("(o p) d -> p o d", p=128))
            nc.sync.dma_start(sn_all[:], sin_d.rearrange("(o p) d -> p o d", p=128))
            for i in range(PATCH // 128):
                sl = slice(i * 128, (i + 1) * 128)
                pt = ps_w.tile([128, 512], f32, tag="ps_w")
                nc.tensor.transpose(pt[:HD, :128], cs_all[:, i, :], ident[:])
                nc.vector.tensor_copy(cosT2[0:64, sl], pt[0:64, :128])
                nc.vector.tensor_copy(cosT2[64:128, sl], pt[0:64, :128])
                pt2 = ps_w.tile([128, 512], f32, tag="ps_w")
                nc.tensor.transpose(pt2[:HD, :128], sn_all[:, i, :], ident[:])
                # rows 0:32 hold +sin[32:64] (read at source partitions 32:64 of
                # q), rows 32:64 hold -sin[0:32]; duplicated for the odd head.
                nc.vector.tensor_copy(sinT2sw[0:32, sl], pt2[32:64, :128])
                nc.vector.tensor_scalar_mul(sinT2sw[32:64, sl], pt2[0:32, :128],
                                            -1.0)
                nc.vector.tensor_copy(sinT2sw[64:96, sl], pt2[32:64, :128])
                nc.vector.tensor_scalar_mul(sinT2sw[96:128, sl], pt2[0:32, :128],
                                            -1.0)

            # --- biases, weights (already bf16; single strided DMA each) ---
            bq_sb = cpool.tile([128, KO], f32)
            nc.sync.dma_start(bq_sb[:], b_d["bq"].rearrange("(o p) -> p o", p=128))
            bo_sb = cpool.tile([128, KO], f32)
            nc.sync.dma_start(bo_sb[:], b_d["bo"].rearrange("(o p) -> p o", p=128))
            bv_bc = cpool.tile([128, H], bf16)
            nc.sync.dma_start(bv_bc[:], b_d["bv"][None, :].to_broadcast((128, H)))

            wb = {}
            for wn in ("wq", "wv", "wk", "wo"):
                wb[wn] = cpool.tile([128, KO, H], bf16, tag=f"wb_{wn}",
                                    name=f"wb_{wn}")
            for wn in ("wq", "wv", "wk", "wo"):
                nc.sync.dma_start(
                    wb[wn][:], w_d[wn].rearrange("(o p) n -> p o n", p=128))

            # ---------------- per batch item ----------------
            def make_item(bi, XT):
                tok0 = bi * T
                QT = ipool.tile([128, KO, T], bf16, tag="QT", name=f"QT_{bi}")
                KT = ipool.tile([128, KO, T], bf16, tag="KT", name=f"KT_{bi}")
                Vst = ipool.tile([128, NJT, NH, HD + 1], bf16, tag="Vst",
                                 name=f"Vst_{bi}")
                AOT = ipool.tile([128, KO, T], bf16, tag="AOT", name=f"AOT_{bi}")

                def emit_vinit():
                    nc.vector.memset(Vst[:, :, :, HD:HD + 1], 1.0)

                pm_state = {}

                def emit_vproj_t(ci, ti, half=None):
                    n0, nw = NCHUNKS[ci]
                    t0, tw = TOK_TILES[ti]
                    kos = (range(KO) if half is None else
                           (range(0, KO // 2) if half == 0
                            else range(KO // 2, KO)))
                    if half in (None, 0):
                        pm_state["v", ci, ti] = ps_w.tile(
                            [128, 512], f32, tag="ps_w",
                            name=f"pmv_{bi}_{ci}_{ti}")
                    pm = pm_state["v", ci, ti]
                    for ko in kos:
                        nc.tensor.matmul(
                            pm[:tw, :nw],
                            XT[:, ko, t0:t0 + tw],
                            wb["wv"][:, ko, n0:n0 + nw],
                            start=(ko == 0), stop=(ko == KO - 1))
                    if half in (None, 1):
                        nc.vector.tensor_tensor(
                            Vst[:tw, ti, n0 // HD:(n0 + nw) // HD, 0:HD],
                            pm[:tw, :nw], bv_bc[:tw, n0:n0 + nw], ADD)

                def emit_qkproj_g(mo, which, ci, half=None):
                    dst, wn, bias = ((QT, "wq", True), (KT, "wk", False))[which]
                    q0, qw = PROJ_CHUNKS[ci]
                    kos = (range(KO) if half is None else
                           (range(0, KO // 2) if half == 0
                            else range(KO // 2, KO)))
                    if half in (None, 0):
                        pm_state["qk", which, mo, ci] = ps_w.tile(
                            [128, 512], f32, tag="ps_w",
                            name=f"pm_{bi}_{wn}_{mo}_{q0}")
                    pm = pm_state["qk", which, mo, ci]
                    for ko in kos:
                        nc.tensor.matmul(
                            pm[:, :qw],
                            wb[wn][:, ko, mo * 128:(mo + 1) * 128],
                            XT[:, ko, q0:q0 + qw],
                            start=(ko == 0), stop=(ko == KO - 1))
                    if half in (None, 1):
                        if bias:
                            nc.vector.tensor_scalar_add(
                                dst[:, mo, q0:q0 + qw], pm[:, :qw],
                                bq_sb[:, mo:mo + 1])
                        else:
                            nc.vector.tensor_copy(dst[:, mo, q0:q0 + qw],
                                                  pm[:, :qw])

                def emit_rope_t(mo, which):
                    tgt = (QT, KT)[which]
                    src = tgt[:, mo, NPREF:T]
                    t1 = rpool.tile([128, PATCH], bf16, tag="rope1")
                    nc.vector.tensor_tensor(t1[:], src, cosT2[:], MUL)
                    t2 = rpool.tile([128, PATCH], bf16, tag="rope2")
                    for (o, sp) in ((0, 32), (32, 0), (64, 96), (96, 64)):
                        nc.vector.tensor_tensor(
                            t2[o:o + 32, :], tgt[sp:sp + 32, mo, NPREF:T],
                            sinT2sw[sp:sp + 32, :], MUL)
                    nc.vector.tensor_tensor(src, t1[:], t2[:], ADD)

                # --- attention, head h: phase A (S + exp into es ring) ---
                def emit_attn_a(h, es_tiles, pump=None):
                    ph = (h % 2) * 64
                    kq = h // 2
                    for ji, (j0, jw) in enumerate(TOK_TILES):
                        if pump is not None:
                            pump()
                        pss = ps_s.tile([128, 1024], f32, tag="ps_s")
                        for qi, (q0, qw) in enumerate(QCHUNKS):
                            nc.tensor.matmul(
                                pss[:jw, q0:q0 + qw],
                                KT[ph:ph + 64, kq, j0:j0 + jw],
                                QT[ph:ph + 64, kq, q0:q0 + qw],
                                start=True, stop=True)
                        es = espool.tile([128, 1024], bf16, tag="es",
                                         name=f"es_{bi}_{h}_{ji}")
                        nc.scalar.activation(es[:jw, :], pss[:jw, :],
                                             FP.Exp, scale=SCALE)
                        es_tiles[ji] = es

                # --- attention, head h: phase B (AV + normalize into nt) ---
                def emit_attn_b(h, es_tiles, nt_tiles, pump=None):
                    ph = (h % 2) * 64
                    for qc in range(2):
                        if pump is not None:
                            pump()
                        av = ps_av.tile([128, 4, HD + 1], f32, tag="av",
                                        name=f"av_{bi}_{h}_{qc}")
                        # ji-major so the PE stream only ever waits on the
                        # most recent exp, not the last one
                        for ji, (j0, jw) in enumerate(TOK_TILES):
                            if pump is not None and ji % 3 == 2:
                                pump()
                            for qtl in range(4):
                                qt = qc * 4 + qtl
                                nc.tensor.matmul(
                                    av[:, qtl, :],
                                    es_tiles[ji][:jw, qt * 128:(qt + 1) * 128],
                                    Vst[:jw, ji, h, :],
                                    start=(ji == 0), stop=(ji == NJT - 1))
                        rc = apool.tile([128, 4], f32, tag="rc")
                        nc.vector.reciprocal(rc[:, :], av[:, :, HD])
                        for qtl in range(4):
                            qt = qc * 4 + qtl
                            nc.vector.tensor_scalar_mul(
                                nt_tiles[qt][:, ph:ph + HD],
                                av[:, qtl, 0:HD], rc[:, qtl:qtl + 1])

                # --- pair epilogue: transpose [q, d-pair] -> AOT feature-major ---
                def emit_pair_fin(kq, nt_tiles, pump=None):
                    pt = ps_pt.tile([128, 2, 128], bf16, tag="pt",
                                    name=f"pt_{bi}_{kq}")
                    for qt in range(0, NQT, 2):
                        if pump is not None:
                            pump()
                        nc.tensor.transpose(pt[:, 0, :], nt_tiles[qt][:, :],
                                            identb[:])
                        nc.tensor.transpose(pt[:, 1, :], nt_tiles[qt + 1][:, :],
                                            identb[:])
                        nc.vector.tensor_copy(
                            AOT[:, kq, qt * 128:(qt + 2) * 128], pt[:, :, :])

                def emit_tail():
                    # 5-query tail for all 16 heads, batched: S packed into one
                    # ps_s slot (heads 0..10 bank A, 11..15 bank B), two exps,
                    # AV accumulated per head into one ps_s slot.
                    qt0, qtw = QTAIL
                    pst = ps_s.tile([128, 1024], f32, tag="ps_s",
                                    name=f"pst_{bi}")
                    nc.vector.memset(pst[:], 0.0)

                    def tcol(h):
                        return (h * qtw * NJT if h <= 10
                                else 512 + (h - 11) * qtw * NJT)

                    for h in range(NH):
                        ph = (h % 2) * 64
                        kq = h // 2
                        for ji, (j0, jw) in enumerate(TOK_TILES):
                            nc.tensor.matmul(
                                pst[:jw,
                                    tcol(h) + ji * qtw: tcol(h) + (ji + 1) * qtw],
                                KT[ph:ph + 64, kq, j0:j0 + jw],
                                QT[ph:ph + 64, kq, qt0:qt0 + qtw],
                                start=True, stop=True)
                    est = apool.tile([128, 1024], bf16, tag="expS",
                                     name=f"est_{bi}")
                    nc.scalar.activation(est[:, 0:495], pst[:, 0:495],
                                         FP.Exp, scale=SCALE)
                    nc.scalar.activation(est[:, 512:737], pst[:, 512:737],
                                         FP.Exp, scale=SCALE)
                    pot = ps_s.tile([128, 1024], f32, tag="ps_s",
                                    name=f"pot_{bi}")
                    for h in range(NH):
                        for ji, (j0, jw) in enumerate(TOK_TILES):
                            nc.tensor.matmul(
                                pot[:HD + 1, h * qtw:(h + 1) * qtw],
                                Vst[:jw, ji, h, :],
                                est[0:jw,
                                    tcol(h) + ji * qtw: tcol(h) + (ji + 1) * qtw],
                                start=(ji == 0), stop=(ji == NJT - 1))
                    rc = apool.tile([1, 512], f32, tag="recip")
                    nc.vector.reciprocal(rc[0:1, :NH * qtw],
                                         pot[64:65, :NH * qtw])
                    rb = apool.tile([64, 512], f32, tag="recipB")
                    nc.gpsimd.partition_broadcast(rb[:, :NH * qtw],
                                                  rc[0:1, :NH * qtw])
                    for h in range(NH):
                        nc.vector.tensor_tensor(
                            AOT[(h % 2) * 64:(h % 2) * 64 + 64, h // 2,
                                qt0:qt0 + qtw],
                            pot[0:64, h * qtw:(h + 1) * qtw],
                            rb[:, h * qtw:(h + 1) * qtw], MUL)

                # --- out-projection, transposed: yT[mo-block, tokens] ---
                def emit_outproj_g(mo, ci, half=None):
                    t0, tw = PROJ_CHUNKS[ci]
                    kos = (range(KO) if half is None else
                           (range(0, KO // 2) if half == 0
                            else range(KO // 2, KO)))
                    if half in (None, 0):
                        pm_state["o", mo, ci] = ps_w.tile(
                            [128, 512], f32, tag="ps_w",
                            name=f"pmo_{bi}_{mo}_{t0}")
                    pm = pm_state["o", mo, ci]
                    for ko in kos:
                        nc.tensor.matmul(
                            pm[:, :tw],
                            wb["wo"][:, ko, mo * 128:(mo + 1) * 128],
                            AOT[:, ko, t0:t0 + tw],
                            start=(ko == 0), stop=(ko == KO - 1))
                    if half in (None, 1):
                        y = ypool.tile([128, 512], f32, tag="y")
                        nc.vector.tensor_scalar_add(y[:, :tw], pm[:, :tw],
                                                    bo_sb[:, mo:mo + 1])
                        nc.sync.dma_start(
                            out_d[mo * 128:(mo + 1) * 128,
                                  tok0 + t0: tok0 + t0 + tw],
                            y[:, :tw])

                def emit_outproj(skip=()):
                    for mo in range(KO):
                        for ci in range(len(PROJ_CHUNKS)):
                            if (mo, ci) not in skip:
                                emit_outproj_g(mo, ci)

                def emit_blocks(extra=None):
                    fills = []

                    def pump():
                        if fills:
                            fills.pop(0)()

                    for kq in range(KO):
                        if kq in (2, 3):
                            tis = range(0, 5) if kq == 2 else range(5, NJT)
                            fills.extend(
                                (lambda ti=ti, half=half:
                                 emit_vproj_t(1, ti, half))
                                for ti in tis for half in range(2))
                        if kq < KO - 1:
                            fills.extend(
                                (lambda kq=kq, which=which, ci=ci, half=half:
                                 emit_qkproj_g(kq + 1, which, ci, half))
                                for which in range(2)
                                for ci in range(len(PROJ_CHUNKS))
                                for half in range(2))
                            fills.append(lambda kq=kq: emit_rope_t(kq + 1, 0))
                            fills.append(lambda kq=kq: emit_rope_t(kq + 1, 1))
                        if extra and kq in extra:
                            fills.extend(extra[kq])
                        nt_tiles = [
                            ntpool.tile([128, 128], bf16, tag="nt",
                                        name=f"nt_{bi}_{kq}_{qt}")
                            for qt in range(NQT)]
                        for h in (2 * kq, 2 * kq + 1):
                            es_tiles = [None] * NJT
                            emit_attn_a(h, es_tiles, pump)
                            emit_attn_b(h, es_tiles, nt_tiles, pump)
                        emit_pair_fin(kq, nt_tiles, pump)
                    while fills:
                        fills.pop(0)()

                def emit_head():
                    emit_vinit()
                    for ti in range(NJT):
                        emit_vproj_t(0, ti)
                    for which in range(2):
                        for ci in range(len(PROJ_CHUNKS)):
                            emit_qkproj_g(0, which, ci)
                    emit_rope_t(0, 0)
                    emit_rope_t(0, 1)

                return {
                    "head": emit_head, "blocks": emit_blocks,
                    "tail": emit_tail, "outproj": emit_outproj,
                    "outproj_g": emit_outproj_g,
                }

            it0 = make_item(0, XT0)
            it0["head"]()
            XT1 = ipool.tile([128, KO, T], bf16, tag="XT", name="XT_1")
            it0["blocks"](extra={7: [lambda: emit_xprep_full(1, XT1)]})
            it0["tail"]()
            it1 = make_item(1, XT1)
            it1["head"]()              # runs during item0 out-proj window
            # defer all of item0's out-proj into item1's blocks as pump fills
            dthunks = [(lambda mo=mo, ci=ci, half=half:
                        it0["outproj_g"](mo, ci, half))
                       for mo in range(KO)
                       for ci in range(len(PROJ_CHUNKS))
                       for half in range(2)]
            it1["blocks"](extra={kq: dthunks[6 * kq:6 * kq + 6]
                                 for kq in range(KO)})
            it1["tail"]()
            it1["outproj"]()

    nc.compile()
    return nc


_NC_CACHE = []
_LAST_RESULT = []


def kernel(hidden_states, cos, sin, wq, bq, wk, wv, bv, wo, bo):
    from concourse.bass_utils import run_bass_kernel_spmd

    def _bf16(x):
        return np.ascontiguousarray(np.asarray(x).astype(ml_dtypes.bfloat16))

    def _f32(x):
        return np.ascontiguousarray(np.asarray(x, dtype=np.float32))

    hs_b = _bf16(hidden_states).reshape(B * T, H)
    shared = {
        "ident": np.eye(128, dtype=ml_dtypes.bfloat16),
        "cos": _f32(cos), "sin": _f32(sin),
        "wq": _bf16(wq), "wk": _bf16(wk), "wv": _bf16(wv), "wo": _bf16(wo),
        "bq": _f32(bq), "bv": _bf16(bv), "bo": _f32(bo),
    }
    if not _NC_CACHE:
        _NC_CACHE.append(build())
    nc = _NC_CACHE[0]

    in_maps = []
    for c in range(NCORES):
        m = dict(shared)
        m["hs"] = np.ascontiguousarray(hs_b[c * TOK:(c + 1) * TOK].T)
        in_maps.append(m)

    try:
        res = run_bass_kernel_spmd(nc, in_maps, core_ids=list(range(NCORES)))
    except Exception:
        # transient NRT device errors (e.g. NRT_EXEC_UNIT_UNRECOVERABLE) have
        # been observed on this fabric; one retry usually succeeds
        time.sleep(2.0)
        res = run_bass_kernel_spmd(nc, in_maps, core_ids=list(range(NCORES)))
    _LAST_RESULT.clear()
    _LAST_RESULT.append(res)
    out = np.concatenate(
        [r["out"].T.reshape(BPC, T, H) for r in res.results], axis=0)
    return out


# revision 21
# speedup vs baseline: 1.3420x; 1.2825x over previous
"""Dinov3 ViT attention kernel for Trainium2 (8 NeuronCores, data-parallel over batch).

Per core: 2 batch items. hidden_states [2*1029, 1024] in, out [2*1029, 1024] f32.

Host pre-casts hidden_states + weights to bf16 (the kernel computes in bf16
internally anyway, so this only halves DMA traffic).

Per item pipeline (PE-dense, interleaved with ACT-bound attention):
  X-prep (strided DMA to feature-major XT) ->
  V-proj chunk 0 (heads 0..7) ->
  for kq in 0..7:  # head pair (2kq, 2kq+1)
    per head: phase A: S^T per key-tile (K=64 matmul) -> exp on ScalarE
              (scale=1/8, no max: |scores| < ~7) into a 9-slot SBUF ring;
              phase B: AV with es as the STATIONARY operand -> out
              [128 queries, 65] per query tile (col 64 = softmax sum via
              ones-augmented V), accumulated over key tiles in one PSUM bank;
              normalize via per-partition reciprocal + tensor_scalar.
    per pair: PE-transpose normalized [q, d] tiles back to feature-major
              AOT, PSUM->SBUF copy on the (otherwise idle) Pool engine.
    (Q/K proj mo+1, RoPE, V chunk 1, prev-item out-proj pumped into gaps.)
  5-query tail batched into one [128,45] PSUM bank + single exp per bank,
  normalized via DVE reciprocal + gpsimd partition_broadcast.
  out-projection emitted transposed: Y^T = Wo^T-stationary @ AOT, bias via
  per-partition tensor_scalar, DMA'd as [H, TOK] f32 (host transposes back).
"""
import sys
import time

sys.path.insert(0, "/opt/trn_rl_repo")

import ml_dtypes
import numpy as np

import concourse.bacc as bacc
import concourse.mybir as mybir
import concourse.tile as tile

f32 = mybir.dt.float32
bf16 = mybir.dt.bfloat16
FP = mybir.ActivationFunctionType
ADD = mybir.AluOpType.add
MUL = mybir.AluOpType.mult

H = 1024
NH = 16
HD = 64
T = 1029
NPREF = 5
PATCH = 1024
B = 16
NCORES = 8
BPC = B // NCORES          # batch items per core
KO = H // 128              # 8 feature k-tiles
TOK = BPC * T              # tokens per core (2058)
SCALE = 1.0 / float(np.sqrt(HD))

TOK_TILES = [(i * 128, min(128, T - i * 128)) for i in range((T + 127) // 128)]
NJT = len(TOK_TILES)                   # 9 key tiles (8 full + 5)
NQT = 8                                # full 128-query tiles (0..1024)
QCHUNKS = [(0, 512), (512, 512)]
QTAIL = (1024, T - 1024)               # 5 queries -> batched-exp path
PROJ_CHUNKS = [(0, 343), (343, 343), (686, 343)]
NCHUNKS = [(0, 512), (512, 512)]


def build():
    nc = bacc.Bacc(None, target_bir_lowering=False)
    hs = nc.dram_tensor("hs", [H, TOK], bf16, kind="ExternalInput")  # host pre-transposed
    cos_d = nc.dram_tensor("cos", [PATCH, HD], f32, kind="ExternalInput")
    sin_d = nc.dram_tensor("sin", [PATCH, HD], f32, kind="ExternalInput")
    w_d = {wn: nc.dram_tensor(wn, [H, H], bf16, kind="ExternalInput")
           for wn in ("wq", "wk", "wv", "wo")}
    b_d = {"bq": nc.dram_tensor("bq", [H], f32, kind="ExternalInput"),
           "bv": nc.dram_tensor("bv", [H], bf16, kind="ExternalInput"),
           "bo": nc.dram_tensor("bo", [H], f32, kind="ExternalInput")}
    ident_d = nc.dram_tensor("ident", [128, 128], bf16, kind="ExternalInput")
    out_d = nc.dram_tensor("out", [H, TOK], f32, kind="ExternalOutput")

    with tile.TileContext(nc) as tc:
        with (
            tc.tile_pool(name="const", bufs=1) as cpool,
            tc.tile_pool(name="item", bufs=1) as ipool,
            tc.tile_pool(name="ao", bufs=2) as aopool,
            tc.tile_pool(name="espool", bufs=9) as espool,
            tc.tile_pool(name="ntpool", bufs=8) as ntpool,
            tc.tile_pool(name="work", bufs=2) as wpool,
            tc.tile_pool(name="rope", bufs=1) as rpool,
            tc.tile_pool(name="attn", bufs=2) as apool,
            tc.tile_pool(name="ypool", bufs=2) as ypool,
            tc.tile_pool(name="ps_s", bufs=2, space="PSUM") as ps_s,
            tc.tile_pool(name="ps_av", bufs=2, space="PSUM") as ps_av,
            tc.tile_pool(name="ps_w", bufs=2, space="PSUM") as ps_w,
        ):
            identb = cpool.tile([128, 128], bf16)
            nc.sync.dma_start(identb[:], ident_d[:])
            ident = cpool.tile([128, 128], f32)
            nc.vector.tensor_copy(ident[:], identb[:])

            # --- X-prep: hs is already feature-major; one strided DMA per item ---
            hs_r = hs.rearrange("(o p) t -> p o t", p=128)

            def emit_xprep_full(bi, XT, nsplit=3):
                step = (T + nsplit - 1) // nsplit
                for t0 in range(0, T, step):
                    tw = min(step, T - t0)
                    nc.sync.dma_start(
                        XT[:, :, t0:t0 + tw],
                        hs_r[:, :, bi * T + t0: bi * T + t0 + tw])

            XT0 = ipool.tile([128, KO, T], bf16, tag="XT", name="XT_0")
            emit_xprep_full(0, XT0)

            # --- cos/sin -> transposed, duplicated, sign-adjusted tables ---
            cosT2 = cpool.tile([128, PATCH], bf16)
            sinT2sw = cpool.tile([128, PATCH], bf16)
            cs_all = rpool.tile([128, PATCH // 128, HD], f32, tag="rope1")
            sn_all = rpool.tile([128, PATCH // 128, HD], f32, tag="rope2")
            nc.sync.dma_start(cs_all[:], cos_d.rearrange("(o p) d -> p o d", p=128))
            nc.sync.dma_start(sn_all[:], sin_d.rearrange("(o p) d -> p o d", p=128))
            for i in range(PATCH // 128):
                sl = slice(i * 128, (i + 1) * 128)
                pt = ps_w.tile([128, 512], f32, tag="ps_w")
                nc.tensor.transpose(pt[:HD, :128], cs_all[:, i, :], ident[:])
                nc.vector.tensor_copy(cosT2[0:64, sl], pt[0:64, :128])
                nc.vector.tensor_copy(cosT2[64:128, sl], pt[0:64, :128])
                pt2 = ps_w.tile([128, 512], f32, tag="ps_w")
                nc.tensor.transpose(pt2[:HD, :128], sn_all[:, i, :], ident[:])
                # rows 0:32 hold +sin[32:64] (read at source partitions 32:64 of
                # q), rows 32:64 hold -sin[0:32]; duplicated for the odd head.
                nc.vector.tensor_copy(sinT2sw[0:32, sl], pt2[32:64, :128])
                nc.vector.tensor_scalar_mul(sinT2sw[32:64, sl], pt2[0:32, :128],
                                            -1.0)
                nc.vector.tensor_copy(sinT2sw[64:96, sl], pt2[32:64, :128])
                nc.vector.tensor_scalar_mul(sinT2sw[96:128, sl], pt2[0:32, :128],
                                            -1.0)

            # --- biases, weights (already bf16; single strided DMA each) ---
            bq_sb = cpool.tile([128, KO], f32)
            nc.sync.dma_start(bq_sb[:], b_d["bq"].rearrange("(o p) -> p o", p=128))
            bo_sb = cpool.tile([128, KO], f32)
            nc.sync.dma_start(bo_sb[:], b_d["bo"].rearrange("(o p) -> p o", p=128))
            bv_bc = cpool.tile([128, H], bf16)
            nc.sync.dma_start(bv_bc[:], b_d["bv"][None, :].to_broadcast((128, H)))

            wb = {}
            for wn in ("wq", "wv", "wk", "wo"):
                wb[wn] = cpool.tile([128, KO, H], bf16, tag=f"wb_{wn}",
                                    name=f"wb_{wn}")
            for wn in ("wq", "wv", "wk", "wo"):
                nc.sync.dma_start(
                    wb[wn][:], w_d[wn].rearrange("(o p) n -> p o n", p=128))

            # ---------------- per batch item ----------------
            def make_item(bi, XT):
                tok0 = bi * T
                QT = ipool.tile([128, KO, T], bf16, tag="QT", name=f"QT_{bi}")
                KT = ipool.tile([128, KO, T], bf16, tag="KT", name=f"KT_{bi}")
                Vst = ipool.tile([128, NJT, NH, HD + 1], bf16, tag="Vst",
                                 name=f"Vst_{bi}")
                AOT = aopool.tile([128, KO, T], bf16, tag="AOT",
                                  name=f"AOT_{bi}")

                def emit_vinit():
                    nc.vector.memset(Vst[:, :, :, HD:HD + 1], 1.0)

                pm_state = {}

                def emit_vproj_t(ci, ti, half=None):
                    n0, nw = NCHUNKS[ci]
                    t0, tw = TOK_TILES[ti]
                    kos = (range(KO) if half is None else
                           (range(0, KO // 2) if half == 0
                            else range(KO // 2, KO)))
                    if half in (None, 0):
                        pm_state["v", ci, ti] = ps_w.tile(
                            [128, 512], f32, tag="ps_w",
                            name=f"pmv_{bi}_{ci}_{ti}")
                    pm = pm_state["v", ci, ti]
                    for ko in kos:
                        nc.tensor.matmul(
                            pm[:tw, :nw],
                            XT[:, ko, t0:t0 + tw],
                            wb["wv"][:, ko, n0:n0 + nw],
                            start=(ko == 0), stop=(ko == KO - 1))
                    if half in (None, 1):
                        nc.vector.tensor_tensor(
                            Vst[:tw, ti, n0 // HD:(n0 + nw) // HD, 0:HD],
                            pm[:tw, :nw], bv_bc[:tw, n0:n0 + nw], ADD)

                def emit_qkproj_g(mo, which, ci, half=None):
                    dst, wn, bias = ((QT, "wq", True), (KT, "wk", False))[which]
                    q0, qw = PROJ_CHUNKS[ci]
                    kos = (range(KO) if half is None else
                           (range(0, KO // 2) if half == 0
                            else range(KO // 2, KO)))
                    if half in (None, 0):
                        pm_state["qk", which, mo, ci] = ps_w.tile(
                            [128, 512], f32, tag="ps_w",
                            name=f"pm_{bi}_{wn}_{mo}_{q0}")
                    pm = pm_state["qk", which, mo, ci]
                    for ko in kos:
                        nc.tensor.matmul(
                            pm[:, :qw],
                            wb[wn][:, ko, mo * 128:(mo + 1) * 128],
                            XT[:, ko, q0:q0 + qw],
                            start=(ko == 0), stop=(ko == KO - 1))
                    if half in (None, 1):
                        if bias:
                            nc.vector.tensor_scalar_add(
                                dst[:, mo, q0:q0 + qw], pm[:, :qw],
                                bq_sb[:, mo:mo + 1])
                        else:
                            nc.vector.tensor_copy(dst[:, mo, q0:q0 + qw],
                                                  pm[:, :qw])

                def emit_rope_t(mo, which):
                    tgt = (QT, KT)[which]
                    src = tgt[:, mo, NPREF:T]
                    t1 = rpool.tile([128, PATCH], bf16, tag="rope1")
                    nc.vector.tensor_tensor(t1[:], src, cosT2[:], MUL)
                    t2 = rpool.tile([128, PATCH], bf16, tag="rope2")
                    for (o, sp) in ((0, 32), (32, 0), (64, 96), (96, 64)):
                        nc.vector.tensor_tensor(
                            t2[o:o + 32, :], tgt[sp:sp + 32, mo, NPREF:T],
                            sinT2sw[sp:sp + 32, :], MUL)
                    nc.vector.tensor_tensor(src, t1[:], t2[:], ADD)

                # --- software-pipelined attention over heads:
                # iteration h: sweep ji emits S/exp(h) interleaved with the
                # AV matmuls of head h-1 (es as stationary), so the ACT
                # engine never drains between heads.
                def emit_norm(h, av_tiles, nt_tiles):
                    ph = (h % 2) * 64
                    for qc in range(2):
                        av = av_tiles[qc]
                        rc = apool.tile([128, 4], f32, tag="rc")
                        nc.vector.reciprocal(rc[:, :], av[:, :, HD])
                        for qtl in range(4):
                            qt = qc * 4 + qtl
                            nc.vector.tensor_scalar_mul(
                                nt_tiles[qt][:, ph:ph + HD],
                                av[:, qtl, 0:HD], rc[:, qtl:qtl + 1])

                # --- pair epilogue: transpose [q, d-pair] -> AOT feature-major ---
                def emit_pair_fin(kq, nt_tiles, pump=None):
                    pt = ps_w.tile([128, 2, 128], bf16, tag="ps_w",
                                   name=f"pt_{bi}_{kq}")
                    for qt in range(0, NQT, 2):
                        if pump is not None:
                            pump()
                        nc.tensor.transpose(pt[:, 0, :], nt_tiles[qt][:, :],
                                            identb[:])
                        nc.tensor.transpose(pt[:, 1, :], nt_tiles[qt + 1][:, :],
                                            identb[:])
                        nc.vector.tensor_copy(
                            AOT[:, kq, qt * 128:(qt + 2) * 128], pt[:, :, :])

                def emit_tail():
                    # 5-query tail for all 16 heads, batched: S packed into one
                    # ps_s slot (heads 0..10 bank A, 11..15 bank B), two exps,
                    # AV accumulated per head into one ps_s slot.
                    qt0, qtw = QTAIL
                    pst = ps_s.tile([128, 1024], f32, tag="ps_s",
                                    name=f"pst_{bi}")
                    nc.vector.memset(pst[:], 0.0)

                    def tcol(h):
                        return (h * qtw * NJT if h <= 10
                                else 512 + (h - 11) * qtw * NJT)

                    for h in range(NH):
                        ph = (h % 2) * 64
                        kq = h // 2
                        for ji, (j0, jw) in enumerate(TOK_TILES):
                            nc.tensor.matmul(
                                pst[:jw,
                                    tcol(h) + ji * qtw: tcol(h) + (ji + 1) * qtw],
                                KT[ph:ph + 64, kq, j0:j0 + jw],
                                QT[ph:ph + 64, kq, qt0:qt0 + qtw],
                                start=True, stop=True)
                    est = apool.tile([128, 1024], bf16, tag="expS", bufs=1,
                                     name=f"est_{bi}")
                    nc.scalar.activation(est[:, 0:495], pst[:, 0:495],
                                         FP.Exp, scale=SCALE)
                    nc.scalar.activation(est[:, 512:737], pst[:, 512:737],
                                         FP.Exp, scale=SCALE)
                    pot = ps_s.tile([128, 1024], f32, tag="ps_s",
                                    name=f"pot_{bi}")
                    for h in range(NH):
                        for ji, (j0, jw) in enumerate(TOK_TILES):
                            nc.tensor.matmul(
                                pot[:HD + 1, h * qtw:(h + 1) * qtw],
                                Vst[:jw, ji, h, :],
                                est[0:jw,
                                    tcol(h) + ji * qtw: tcol(h) + (ji + 1) * qtw],
                                start=(ji == 0), stop=(ji == NJT - 1))
                    rc = apool.tile([1, NH * qtw], f32, tag="recip", bufs=1)
                    nc.vector.reciprocal(rc[0:1, :NH * qtw],
                                         pot[64:65, :NH * qtw])
                    rb = apool.tile([64, NH * qtw], f32, tag="recipB", bufs=1)
                    nc.gpsimd.partition_broadcast(rb[:, :NH * qtw],
                                                  rc[0:1, :NH * qtw])
                    for h in range(NH):
                        nc.vector.tensor_tensor(
                            AOT[(h % 2) * 64:(h % 2) * 64 + 64, h // 2,
                                qt0:qt0 + qtw],
                            pot[0:64, h * qtw:(h + 1) * qtw],
                            rb[:, h * qtw:(h + 1) * qtw], MUL)

                # --- out-projection, transposed: yT[mo-block, tokens] ---
                def emit_outproj_g(mo, ci, half=None):
                    t0, tw = PROJ_CHUNKS[ci]
                    kos = (range(KO) if half is None else
                           (range(0, KO // 2) if half == 0
                            else range(KO // 2, KO)))
                    if half in (None, 0):
                        pm_state["o", mo, ci] = ps_w.tile(
                            [128, 512], f32, tag="ps_w",
                            name=f"pmo_{bi}_{mo}_{t0}")
                    pm = pm_state["o", mo, ci]
                    for ko in kos:
                        nc.tensor.matmul(
                            pm[:, :tw],
                            wb["wo"][:, ko, mo * 128:(mo + 1) * 128],
                            AOT[:, ko, t0:t0 + tw],
                            start=(ko == 0), stop=(ko == KO - 1))
                    if half in (None, 1):
                        y = ypool.tile([128, 512], f32, tag="y")
                        nc.vector.tensor_scalar_add(y[:, :tw], pm[:, :tw],
                                                    bo_sb[:, mo:mo + 1])
                        nc.sync.dma_start(
                            out_d[mo * 128:(mo + 1) * 128,
                                  tok0 + t0: tok0 + t0 + tw],
                            y[:, :tw])

                def emit_outproj(skip=()):
                    for mo in range(KO):
                        for ci in range(len(PROJ_CHUNKS)):
                            if (mo, ci) not in skip:
                                emit_outproj_g(mo, ci)

                def emit_blocks(extra=None):
                    # fills: list of (prio_pair, thunk); prio_pair = pair
                    # index whose S-matmuls REQUIRE this fill to be emitted
                    # first (QT/KT writers), or None for order-free work.
                    fills = []

                    def pump():
                        if fills:
                            fills.pop(0)[1]()

                    def drain_required(kq):
                        i = 0
                        while i < len(fills):
                            p, th = fills[i]
                            if p is not None and p <= kq:
                                fills.pop(i)[1]()
                            else:
                                i += 1

                    def enqueue(kq):
                        if kq in (2, 3):
                            tis = range(0, 5) if kq == 2 else range(5, NJT)
                            fills.extend(
                                (None, lambda ti=ti: emit_vproj_t(1, ti))
                                for ti in tis)
                        if kq < KO - 1:
                            fills.extend(
                                (kq + 1,
                                 lambda kq=kq, which=which, ci=ci:
                                 emit_qkproj_g(kq + 1, which, ci))
                                for which in range(2)
                                for ci in range(len(PROJ_CHUNKS)))
                            fills.append(
                                (kq + 1, lambda kq=kq: emit_rope_t(kq + 1, 0)))
                            fills.append(
                                (kq + 1, lambda kq=kq: emit_rope_t(kq + 1, 1)))
                        if extra and kq in extra:
                            fills.extend((None, th) for th in extra[kq])

                    es_prev = [None] * NJT   # es ring of head h-1
                    es_cur = [None] * NJT
                    av_hist = {}             # head -> av tiles
                    nt_pair = {}             # pair -> nt tiles
                    for h in range(NH + 2):
                        kq = h // 2
                        # norms + pair epilogue for head h-2 (av slots about
                        # to be re-used by head h-1's allocations below)
                        if 2 <= h <= NH + 1:
                            hh = h - 2
                            if hh % 2 == 0:
                                nt_pair[hh // 2] = [
                                    ntpool.tile([128, 128], bf16, tag="nt",
                                                name=f"nt_{bi}_{hh//2}_{qt}")
                                    for qt in range(NQT)]
                            emit_norm(hh, av_hist.pop(hh), nt_pair[hh // 2])
                            if hh % 2 == 1:
                                emit_pair_fin(hh // 2, nt_pair.pop(hh // 2),
                                              pump)
                        if h < NH:
                            if h % 2 == 0:
                                enqueue(kq)
                                drain_required(kq)
                            ph = (h % 2) * 64
                        # av tiles for head h-1 (written during this sweep)
                        if 1 <= h <= NH:
                            av_hist[h - 1] = [
                                ps_av.tile([128, 4, HD + 1], f32, tag="av",
                                           name=f"av_{bi}_{h-1}_{qc}")
                                for qc in range(2)]
                        for ji, (j0, jw) in enumerate(TOK_TILES):
                            pump()
                            if h < NH:
                                pss = ps_s.tile([128, 1024], f32, tag="ps_s")
                                for qi, (q0, qw) in enumerate(QCHUNKS):
                                    nc.tensor.matmul(
                                        pss[:jw, q0:q0 + qw],
                                        KT[ph:ph + 64, kq, j0:j0 + jw],
                                        QT[ph:ph + 64, kq, q0:q0 + qw],
                                        start=True, stop=True)
                            if 1 <= h <= NH:
                                av = av_hist[h - 1]
                                for qt in range(NQT):
                                    nc.tensor.matmul(
                                        av[qt // 4][:, qt % 4, :],
                                        es_prev[ji][:jw,
                                                    qt * 128:(qt + 1) * 128],
                                        Vst[:jw, ji, h - 1, :],
                                        start=(ji == 0), stop=(ji == NJT - 1))
                            if h < NH:
                                es = espool.tile([128, 1024], bf16, tag="es",
                                                 name=f"es_{bi}_{h}_{ji}")
                                nc.scalar.activation(es[:jw, :], pss[:jw, :],
                                                     FP.Exp, scale=SCALE)
                                es_cur[ji] = es
                        es_prev, es_cur = es_cur, [None] * NJT
                    while fills:
                        fills.pop(0)[1]()

                def emit_head():
                    emit_vinit()
                    for ti in range(NJT):
                        emit_vproj_t(0, ti)
                    for which in range(2):
                        for ci in range(len(PROJ_CHUNKS)):
                            emit_qkproj_g(0, which, ci)
                    emit_rope_t(0, 0)
                    emit_rope_t(0, 1)

                return {
                    "head": emit_head, "blocks": emit_blocks,
                    "tail": emit_tail, "outproj": emit_outproj,
                    "outproj_g": emit_outproj_g,
                }

            it0 = make_item(0, XT0)
            it0["head"]()
            XT1 = ipool.tile([128, KO, T], bf16, tag="XT", name="XT_1")
            it0["blocks"](extra={7: [lambda: emit_xprep_full(1, XT1)]})
            it0["tail"]()
            it1 = make_item(1, XT1)
            it1["head"]()              # runs during item0 out-proj window
            # defer all of item0's out-proj into item1's blocks as pump fills
            dthunks = [(lambda mo=mo, ci=ci: it0["outproj_g"](mo, ci))
                       for mo in range(KO)
                       for ci in range(len(PROJ_CHUNKS))]
            it1["blocks"](extra={kq: dthunks[3 * kq:3 * kq + 3]
                                 for kq in range(KO)})
            it1["tail"]()
            it1["outproj"]()

    nc.compile()
    return nc


_NC_CACHE = []
_LAST_RESULT = []


def kernel(hidden_states, cos, sin, wq, bq, wk, wv, bv, wo, bo):
    from concourse.bass_utils import run_bass_kernel_spmd

    def _bf16(x):
        return np.ascontiguousarray(np.asarray(x).astype(ml_dtypes.bfloat16))

    def _f32(x):
        return np.ascontiguousarray(np.asarray(x, dtype=np.float32))

    hs_b = _bf16(hidden_states).reshape(B * T, H)
    shared = {
        "ident": np.eye(128, dtype=ml_dtypes.bfloat16),
        "cos": _f32(cos), "sin": _f32(sin),
        "wq": _bf16(wq), "wk": _bf16(wk), "wv": _bf16(wv), "wo": _bf16(wo),
        "bq": _f32(bq), "bv": _bf16(bv), "bo": _f32(bo),
    }
    if not _NC_CACHE:
        _NC_CACHE.append(build())
    nc = _NC_CACHE[0]

    in_maps = []
    for c in range(NCORES):
        m = dict(shared)
        m["hs"] = np.ascontiguousarray(hs_b[c * TOK:(c + 1) * TOK].T)
        in_maps.append(m)

    try:
        res = run_bass_kernel_spmd(nc, in_maps, core_ids=list(range(NCORES)))
    except Exception:
        # transient NRT device errors (e.g. NRT_EXEC_UNIT_UNRECOVERABLE) have
        # been observed on this fabric; one retry usually succeeds
        time.sleep(2.0)
        res = run_bass_kernel_spmd(nc, in_maps, core_ids=list(range(NCORES)))
    _LAST_RESULT.clear()
    _LAST_RESULT.append(res)
    out = np.concatenate(
        [r["out"].T.reshape(BPC, T, H) for r in res.results], axis=0)
    return out
